# revision 1
# baseline (speedup 1.0000x reference)
"""ExemplarAttention Trainium2 kernel (8 NeuronCores, batch-sharded).

logits[b,c] = gamma * log(sum_{n:label[n]=c} exp(-beta * sum_k w_k (x[b,k]-e[n,k])^2) + eps)

Strategy:
  - Data-parallel over batch B=1024 across 8 cores (128 rows/core = one partition tile).
  - Host precomputes the tiny constrained params (softmax(w), beta, gamma),
    x^2@w (per-row bias), e^2@w, and sorts exemplars by class label so the
    per-class scatter-add becomes contiguous segment sums.
  - On device, per 2048-column PSUM super-tile:
      psum = (ones x -S*e2w/2)            [K=1 bf16 matmul, start=True]
           + S * sum_k xw_t[k].T @ e_t[k] [fp8 DoubleRow matmuls, 2 groups x K=256]
    i.e. psum[m,n] = S * (cross[m,n] - e2w[n]/2).   (S rescales x*w into fp8 range)
  - ScalarE: exp((2*beta/S)*psum + (-beta*x2w)[m]) per class-segment piece with
    accum_out -> per-class partial sums directly (no one-hot GEMM, no transpose).
  - Tail: one 3D tensor_reduce merges the piece partials, Ln(+1e-9), *gamma, DMA out.
"""

import os
from contextlib import ExitStack

import numpy as np

B, N, D, C = 1024, 16384, 512, 10
NCORES = 8
B_LOC = B // NCORES          # 128
NG = 2                       # DoubleRow groups (K=256 each)
SUPER = 2048                 # psum super-tile width (4 banks)
NSUPER = N // SUPER
NTILE = 512                  # matmul free dim (1 psum bank)
EPS = 1e-9
S_SCALE = 128.0              # fp8 scale applied to x*w (and the e2w aug row)

# e_t DMA blocks: (col_start, width), two supers each. Coarse blocks keep the
# number of PE wait-points low (frequent micro-waits make the PE's HAM clock
# gate oscillate between 1.2 and 2.4 GHz, halving matmul throughput).
ET_BLOCKS = [(c, 2 * SUPER) for c in range(0, N, 2 * SUPER)]
# PE warmup matmuls issued before the main stream: they keep the PE busy for
# the HAM SHORT window (~3.4us) while giving the e_t DMA stream a head start
# so the matmul stream never catches the DMA stream (which would micro-stall
# the PE and re-throttle the clock gate).
N_WARMUP_MM = 14

_prog_cache = {}


def _np_dt(mybir, name):
    return mybir.dt.np(getattr(mybir.dt, name))


def _compute_pieces(counts):
    """Split each class's sorted-exemplar segment at SUPER boundaries.

    Returns (pieces, maxp): pieces is a list of (super_idx, cls, piece_idx,
    g0, g1) with global column range [g0, g1)."""
    starts = np.concatenate([[0], np.cumsum(counts)]).astype(int)
    pieces = []
    piece_counter = [0] * C
    for c in range(C):
        g0, g1 = int(starts[c]), int(starts[c + 1])
        while g0 < g1:
            end = min(g1, (g0 // SUPER + 1) * SUPER)
            pieces.append((g0 // SUPER, c, piece_counter[c], g0, end))
            piece_counter[c] += 1
            g0 = end
    maxp = max(piece_counter) if max(piece_counter) > 0 else 1
    return pieces, maxp


def _build_program(pieces, maxp, beta, gamma):
    import concourse.bass as bass  # noqa: F401
    import concourse.tile as tile
    from concourse import bacc, mybir

    fp8 = mybir.dt.float8e4
    bf16 = mybir.dt.bfloat16
    f32 = mybir.dt.float32

    nc = bacc.Bacc("TRN2", target_bir_lowering=False, debug=False,
                   num_devices=NCORES)

    e_t_d = nc.dram_tensor("e_t", [NG, 128, 2, N], fp8, kind="ExternalInput").ap()
    xw_t_d = nc.dram_tensor("xw_t", [128, NG, 2, B_LOC], fp8,
                            kind="ExternalInput").ap()
    aug_d = nc.dram_tensor("aug", [1, N + 128], bf16, kind="ExternalInput").ap()
    bias_d = nc.dram_tensor("bias", [B_LOC, 1], f32, kind="ExternalInput").ap()
    out_d = nc.dram_tensor("logits", [B_LOC, C], f32, kind="ExternalOutput").ap()

    act_scale = float(2.0 * beta / S_SCALE)

    by_super = [[] for _ in range(NSUPER)]
    for s, c, p, g0, g1 in pieces:
        by_super[s].append((c, p, g0, g1))

    # super -> (block index, col offset within block)
    sup_block = {}
    for bi, (c0, w) in enumerate(ET_BLOCKS):
        for s in range(c0 // SUPER, (c0 + w) // SUPER):
            sup_block[s] = (bi, s * SUPER - c0)

    with tile.TileContext(nc) as tc, ExitStack() as ctx:
        singles = ctx.enter_context(tc.tile_pool(name="singles", bufs=1))
        et_pool = ctx.enter_context(tc.tile_pool(name="et", bufs=len(ET_BLOCKS) * NG))
        psum_pool = ctx.enter_context(tc.tile_pool(name="ps", bufs=2, space="PSUM"))
        sc_pool = ctx.enter_context(tc.tile_pool(name="sc", bufs=2))

        # Dummy activation first so the ACT table load runs during the DMA
        # startup window instead of blocking the first real exp.
        dummy = singles.tile([128, 1], f32)
        nc.vector.memset(dummy[:, :], 0.0)
        nc.scalar.activation(out=dummy[:, :], in_=dummy[:, :],
                             func=mybir.ActivationFunctionType.Exp, scale=1.0)

        # aug row (-S*e2w/2) + ones row for the K=1 psum pre-fill matmuls.
        aug_sb = singles.tile([1, N + 128], bf16)
        nc.sync.dma_start(out=aug_sb[:, :], in_=aug_d[:, :])
        bias_sb = singles.tile([B_LOC, 1], f32)
        nc.scalar.dma_start(out=bias_sb[:, :], in_=bias_d[:, :])

        et_tiles = {}
        dma_engines = [nc.sync, nc.scalar]
        di = 0
        for bi, (c0, w) in enumerate(ET_BLOCKS):
            for g in range(NG):
                et_tiles[(bi, g)] = et_pool.tile(
                    [128, 2, 2 * SUPER], fp8, tag="et", name=f"et{bi}_{g}")

        def load_et(bi, g, eng=None):
            nonlocal di
            c0, w = ET_BLOCKS[bi]
            (eng or dma_engines[di % len(dma_engines)]).dma_start(
                out=et_tiles[(bi, g)][:, :, :w], in_=e_t_d[g, :, :, c0:c0 + w])
            di += 1

        # Block 0 rides at the head of both rings so super 0's matmuls can
        # start as soon as possible.
        load_et(0, 0, nc.sync)
        load_et(0, 1, nc.scalar)

        def emit_aug(s, ps):
            for j in range(SUPER // NTILE):
                cs = slice(j * NTILE, (j + 1) * NTILE)
                gcs = slice(s * SUPER + j * NTILE, s * SUPER + (j + 1) * NTILE)
                nc.tensor.matmul(ps[:, cs], lhsT=aug_sb[:, N:N + B_LOC],
                                 rhs=aug_sb[:, gcs], start=True, stop=False)

        # Warmup + hoisted aug matmuls for supers 0/1: they only need aug_sb,
        # so they run during the e_t DMA window — prefilling PSUM, warming
        # the PE clock gate (HAM), and giving the DMA stream a head start.
        ps_pre = [psum_pool.tile([128, SUPER], f32, tag="ps", name=f"ps{s}")
                  for s in range(2)]
        # Warmup operands come from a memset tile so the warmup matmuls have
        # no DMA dependency: full-array (K=128) PE activity starts right
        # after the preamble, opens the HAM clock gate, and intentionally
        # delays the main stream until the e_t DMA has an uncatchable lead
        # (a main stream that catches the DMA micro-stalls and re-throttles
        # the PE clock to 1.2 GHz).
        dmy = singles.tile([128, B_LOC + NTILE], bf16)
        nc.vector.memset(dmy[:, :], 0.0)
        for _ in range(N_WARMUP_MM):
            nc.tensor.matmul(ps_pre[0][:, 0:NTILE], lhsT=dmy[:, 0:B_LOC],
                             rhs=dmy[:, B_LOC:], start=True, stop=True)
        for s in (0, 1):
            emit_aug(s, ps_pre[s])

        # x*w weights (tiny) ride the scalar ring behind bias.
        xw_sb = singles.tile([128, NG, 2, B_LOC], fp8)
        nc.scalar.dma_start(out=xw_sb[:, :, :, :], in_=xw_t_d[:, :, :, :])

        acc = singles.tile([128, C * maxp], f32)
        nc.vector.memset(acc[:, :], 0.0)
        eps_sb = singles.tile([128, 1], f32)
        nc.vector.memset(eps_sb[:, :], float(EPS))

        for bi in range(1, len(ET_BLOCKS)):
            for g in range(NG):
                load_et(bi, g)

        for s in range(NSUPER):
            bi, off = sup_block[s]
            if s < 2:
                ps = ps_pre[s]
            else:
                ps = psum_pool.tile([128, SUPER], f32, tag="ps", name=f"ps{s}")
                emit_aug(s, ps)
            # DoubleRow main matmuls, k-major so weights reload once per group
            for g in range(NG):
                et = et_tiles[(bi, g)]
                for j in range(SUPER // NTILE):
                    cs = slice(j * NTILE, (j + 1) * NTILE)
                    ecs = slice(off + j * NTILE, off + (j + 1) * NTILE)
                    nc.tensor.matmul(
                        ps[:, cs], lhsT=xw_sb[:, g, :, :],
                        rhs=et[:, :, ecs], start=False, stop=(g == NG - 1),
                        perf_mode=mybir.MatmulPerfMode.DoubleRow)

            # One wide exp per super on ScalarE; the per-class segment sums
            # run on the otherwise-idle VectorE from the f32 scratch.
            sc = sc_pool.tile([128, SUPER], f32, tag="sc")
            nc.scalar.activation(
                out=sc[:, :],
                in_=ps[:, :],
                func=mybir.ActivationFunctionType.Exp,
                bias=bias_sb[:, :],
                scale=act_scale,
            )
            for c, p, g0, g1 in by_super[s]:
                l0, l1 = g0 - s * SUPER, g1 - s * SUPER
                nc.vector.tensor_reduce(
                    out=acc[:, c * maxp + p:c * maxp + p + 1],
                    in_=sc[:, l0:l1],
                    axis=mybir.AxisListType.X,
                    op=mybir.AluOpType.add,
                )

        class_sum = singles.tile([128, C], f32)
        nc.vector.tensor_reduce(
            out=class_sum[:, :],
            in_=acc.rearrange("q (c m) -> q c m", c=C),
            axis=mybir.AxisListType.X,
            op=mybir.AluOpType.add,
        )
        logits_sb = singles.tile([128, C], f32)
        nc.scalar.activation(
            out=logits_sb[:, :],
            in_=class_sum[:, :],
            func=mybir.ActivationFunctionType.Ln,
            bias=eps_sb[:, :],
            scale=1.0,
        )
        nc.vector.tensor_scalar_mul(logits_sb[:, :], logits_sb[:, :], float(gamma))
        nc.sync.dma_start(out=out_d[:, :], in_=logits_sb[:, :])

    nc.compile()

    # Both Exp and Ln live in act-func-set 6 (natural_log_exp_and_others);
    # the insertion pass picks per-func sets and pays a mid-kernel reload.
    # Point the first load at set 6 and drop the now-redundant extras.
    loads = [(b, i) for b in nc.main_func.blocks for i in b.instructions
             if isinstance(i, mybir.InstLoadActFuncSet)]
    if loads:
        loads[0][1].act_func_set_id = 6
        for b, i in loads[1:]:
            if i.sync_info is None or (
                    not i.sync_info.on_wait and not i.sync_info.on_update):
                b.instructions.remove(i)
            else:
                i.act_func_set_id = 6
    return nc


def _prepare(x, ex_feats, ex_labels, w_unconstrained, gamma_unconstrained,
             beta_unconstrained):
    from concourse import mybir

    x = np.asarray(x, dtype=np.float64)
    e = np.asarray(ex_feats, dtype=np.float64)
    labels = np.asarray(ex_labels).astype(np.int64)
    wu = np.asarray(w_unconstrained, dtype=np.float64)

    beta = float(np.log1p(np.exp(np.float64(beta_unconstrained)))) + EPS
    gamma = float(np.log1p(np.exp(np.float64(gamma_unconstrained)))) + EPS
    wexp = np.exp(wu - wu.max())
    w = wexp / wexp.sum() + EPS

    perm = np.argsort(labels, kind="stable")
    e_sorted = e[perm]
    counts = np.bincount(labels[perm], minlength=C)

    bf16 = _np_dt(mybir, "bfloat16")
    fp8 = _np_dt(mybir, "float8e4")

    # e_t[g, r, s, n] = e_sorted[n, (2g+s)*128 + r]
    e_t = np.ascontiguousarray(
        e_sorted.T.reshape(NG, 2, 128, N).transpose(0, 2, 1, 3)).astype(fp8)

    xw = x * w[None, :]                               # (B, D)
    x2w = (x * x) @ w                                 # (B,)
    e2w = (e_sorted * e_sorted) @ w                   # (N,)

    aug = np.zeros((1, N + 128), dtype=bf16)
    aug[0, :N] = (-0.5 * S_SCALE * e2w).astype(bf16)
    aug[0, N:] = np.ones(128, dtype=bf16)

    per_core = []
    for cid in range(NCORES):
        rows = slice(cid * B_LOC, (cid + 1) * B_LOC)
        xw_c = S_SCALE * xw[rows]                     # (128, 512)
        # xw_t[r, g, s, m] = S * xw_c[m, (2g+s)*128+r]
        xw_t = np.ascontiguousarray(
            xw_c.T.reshape(NG, 2, 128, B_LOC).transpose(2, 0, 1, 3)).astype(fp8)
        bias_c = (-beta * x2w[rows]).astype(np.float32).reshape(B_LOC, 1)
        per_core.append({
            "e_t": e_t,
            "xw_t": xw_t,
            "aug": aug,
            "bias": bias_c,
        })
    return per_core, counts, beta, gamma


def kernel(x, ex_feats, ex_labels, w_unconstrained, gamma_unconstrained,
           beta_unconstrained, _want_results=False, **run_kwargs):
    from concourse.bass_utils import run_bass_kernel_spmd

    per_core, counts, beta, gamma = _prepare(
        x, ex_feats, ex_labels, w_unconstrained, gamma_unconstrained,
        beta_unconstrained)

    pieces, maxp = _compute_pieces(counts)
    key = (tuple(pieces), maxp, round(beta, 12), round(gamma, 12))
    if key not in _prog_cache:
        _prog_cache[key] = _build_program(pieces, maxp, beta, gamma)
    nc = _prog_cache[key]

    res = run_bass_kernel_spmd(nc, per_core, list(range(NCORES)), **run_kwargs)
    out = np.concatenate(
        [np.asarray(res.results[cid]["logits"], dtype=np.float32)
         for cid in range(NCORES)], axis=0)
    if _want_results:
        return out, res
    return out



# revision 7
# speedup vs baseline: 1.3280x; 1.3280x over previous
"""ExemplarAttention Trainium2 kernel (8 NeuronCores, exemplar-sharded).

logits[b,c] = gamma * log(sum_{n:label[n]=c} exp(-beta * sum_k w_k (x[b,k]-e[n,k])^2) + eps)

Strategy (transposed layout, exemplars on partitions):
  - Shard the N=16384 exemplar bank across 8 cores (2048 each, 16 tiles of
    128); replicate the batch side. Per n-tile the PE computes
    psum[n, m] = S * cross[n, m] with e-features as the stationary operand
    (fp8 DoubleRow, 2 groups of K=256) and S*x*w as the moving operand
    (m = full B = 1024 free columns, so LDWEIGHTS amortizes).
  - ScalarE: sim[n, m] = exp((2*beta/S)*psum + (-beta*e2w)[n]) -- e2w is a
    per-partition bias, so no PSUM-prefill matmuls are needed at all. The
    exp(-beta*x2w[m]) factor comes out of the log and is applied on host.
  - Class scatter-add runs on the PE as a one-hot matmul: sim is written in
    fp8 and consumed as a DoubleRow moving operand, so each K=256 pass sums
    two n-tiles: cls_psum[c, m] += onehot[n, c].T @ sim[n, m]. Half a pass
    per column instead of the 1 full pass a PSUM-prefill (or a DVE
    segment-reduce, which is 1x-rate only) would cost.
  - Each core DMAs out its partial class sums [16, 1024]; the host sums the
    8 partials (the unshard step for an associative segment-sum), multiplies
    by exp(-beta*x2w), and applies gamma*log(.+eps) in float64.
"""

import os
from contextlib import ExitStack

import numpy as np

B, N, D, C = 1024, 16384, 512, 10
NCORES = 8
N_LOC = N // NCORES          # 2048 exemplars per core
NTILES = N_LOC // 128        # 16 n-tiles of 128 exemplars
NPAIR = NTILES // 2          # 8 DoubleRow pairs for the one-hot matmul
NG = 2                       # DoubleRow groups over D=512 (K=256 each)
MT = 512                     # matmul free-dim chunk (1 psum bank)
CP = 16                      # one-hot columns padded 10 -> 16 (DR step%16)
EPS = 1e-9
S_SCALE = 128.0              # fp8 scale applied to x*w
# PE warmup matmuls (no DMA deps): keep the PE busy from the end of the
# preamble until xw/e_t arrive so the HAM clock gate opens early.
N_WARMUP_MM = 6

_prog_cache = {}


def _np_dt(mybir, name):
    return mybir.dt.np(getattr(mybir.dt, name))


def _build_program(beta):
    import concourse.bass as bass  # noqa: F401
    import concourse.tile as tile
    from concourse import bacc, mybir

    fp8 = mybir.dt.float8e4
    bf16 = mybir.dt.bfloat16
    f32 = mybir.dt.float32

    nc = bacc.Bacc("TRN2", target_bir_lowering=False, debug=False,
                   num_devices=NCORES)

    # e_t[g, r, s, n] = e[n_glob, (2g+s)*128 + r]   (stationary operand)
    e_t_d = nc.dram_tensor("e_t", [NG, 128, 2, N_LOC], fp8,
                           kind="ExternalInput").ap()
    # xw_t[r, g, s, m] = S * x[m, (2g+s)*128 + r] * w[...]  (moving operand)
    xw_t_d = nc.dram_tensor("xw_t", [128, NG, 2, B], fp8,
                            kind="ExternalInput").ap()
    # oh[r, s, p*CP + c] = 1 if label[(2p+s)*128 + r] == c
    oh_d = nc.dram_tensor("oh", [128, 2, NPAIR * CP], fp8,
                          kind="ExternalInput").ap()
    # bias[r, t] = -beta * e2w[t*128 + r]
    bias_d = nc.dram_tensor("bias", [128, NTILES], f32,
                            kind="ExternalInput").ap()
    out_d = nc.dram_tensor("cls", [CP, B], f32, kind="ExternalOutput").ap()

    act_scale = float(2.0 * beta / S_SCALE)
    DR = mybir.MatmulPerfMode.DoubleRow

    with tile.TileContext(nc) as tc, ExitStack() as ctx:
        singles = ctx.enter_context(tc.tile_pool(name="singles", bufs=1))
        sim_pool = ctx.enter_context(tc.tile_pool(name="sim", bufs=3))
        psum_pool = ctx.enter_context(tc.tile_pool(name="ps", bufs=3,
                                                   space="PSUM"))
        cls_pool = ctx.enter_context(tc.tile_pool(name="cls", bufs=1,
                                                  space="PSUM"))

        # Dummy activation so the ACT exp-table load happens during the DMA
        # startup window instead of blocking the first real exp.
        dummy = singles.tile([128, 1], f32)
        nc.vector.memset(dummy[:, :], 0.0)
        nc.scalar.activation(out=dummy[:, :], in_=dummy[:, :],
                             func=mybir.ActivationFunctionType.Exp, scale=1.0)

        # Warmup matmuls from a memset tile: full-array PE activity right
        # after the preamble opens the HAM clock gate while the DMAs run.
        dmy = singles.tile([128, 128 + MT], bf16)
        nc.vector.memset(dmy[:, :], 0.0)
        ps_w = psum_pool.tile([128, 1024], f32, tag="ps", name="ps_w")
        for _ in range(N_WARMUP_MM):
            nc.tensor.matmul(ps_w[:, 0:MT], lhsT=dmy[:, 0:128],
                             rhs=dmy[:, 128:], start=True, stop=True)

        # DMA rings: scalar carries xw (needed first) + small tensors,
        # sync carries the e_t stream in 4 chunks per group so the first
        # n-tiles are ready as early as possible.
        xw_sb = singles.tile([128, NG, 2, B], fp8)
        nc.scalar.dma_start(out=xw_sb[:, :, :, :], in_=xw_t_d[:, :, :, :])
        bias_sb = singles.tile([128, NTILES], f32)
        nc.scalar.dma_start(out=bias_sb[:, :], in_=bias_d[:, :])
        oh_sb = singles.tile([128, 2, NPAIR * CP], fp8)
        nc.scalar.dma_start(out=oh_sb[:, :, :], in_=oh_d[:, :, :])

        # Each e_t chunk is its own tile so early n-tiles don't wait for the
        # whole stream (Tile tracks dependencies at tile granularity).
        ECH = 4                      # chunks per group
        CW = N_LOC // ECH
        et_sb = {}
        for ci in range(ECH):
            cs = slice(ci * CW, (ci + 1) * CW)
            for g in range(NG):
                et_sb[(g, ci)] = singles.tile([128, 2, CW],
                                              fp8, name=f"et{g}_{ci}")
                nc.sync.dma_start(out=et_sb[(g, ci)][:, :, :],
                                  in_=e_t_d[g, :, :, cs])

        cls_ps = cls_pool.tile([128, B], f32, name="cls_ps")

        def emit_oh(p, sim_sb):
            for j in range(B // MT):
                cs = slice(j * MT, (j + 1) * MT)
                nc.tensor.matmul(
                    cls_ps[0:CP, cs],
                    lhsT=oh_sb[:, :, p * CP:(p + 1) * CP],
                    rhs=sim_sb[:, :, cs],
                    start=(p == 0), stop=(p == NPAIR - 1),
                    perf_mode=DR)

        # One-hot class-sum matmuls trail the cross matmuls by one pair so
        # the PE never waits on ScalarE's exp (which lags ~1us behind).
        sim_tiles = []
        for p in range(NPAIR):
            sim_sb = sim_pool.tile([128, 2, B], fp8, tag="sim")
            sim_tiles.append(sim_sb)
            for s in range(2):
                t = 2 * p + s
                ps = psum_pool.tile([128, B], f32, tag="ps", name=f"ps{t}")
                for g in range(NG):
                    ci, off = divmod(128 * t, CW)
                    lhsT = et_sb[(g, ci)][:, :, off:off + 128]
                    for j in range(B // MT):
                        cs = slice(j * MT, (j + 1) * MT)
                        nc.tensor.matmul(
                            ps[:, cs], lhsT=lhsT,
                            rhs=xw_sb[:, g, :, cs],
                            start=(g == 0), stop=(g == NG - 1),
                            perf_mode=DR)
                nc.scalar.activation(
                    out=sim_sb[:, s, :],
                    in_=ps[:, :],
                    func=mybir.ActivationFunctionType.Exp,
                    bias=bias_sb[:, t:t + 1],
                    scale=act_scale,
                )
            if p >= 1:
                emit_oh(p - 1, sim_tiles[p - 1])
        emit_oh(NPAIR - 1, sim_tiles[NPAIR - 1])

        cls_out = singles.tile([CP, B], f32)
        nc.vector.tensor_copy(cls_out[:, :], cls_ps[0:CP, :])
        nc.sync.dma_start(out=out_d[:, :], in_=cls_out[:, :])

    nc.compile()
    return nc


def _prepare(x, ex_feats, ex_labels, w_unconstrained, gamma_unconstrained,
             beta_unconstrained):
    from concourse import mybir

    x = np.asarray(x, dtype=np.float64)
    e = np.asarray(ex_feats, dtype=np.float64)
    labels = np.asarray(ex_labels).astype(np.int64)
    wu = np.asarray(w_unconstrained, dtype=np.float64)

    beta = float(np.log1p(np.exp(np.float64(beta_unconstrained)))) + EPS
    gamma = float(np.log1p(np.exp(np.float64(gamma_unconstrained)))) + EPS
    wexp = np.exp(wu - wu.max())
    w = wexp / wexp.sum() + EPS

    fp8 = _np_dt(mybir, "float8e4")
    bf16 = _np_dt(mybir, "bfloat16")

    xw = (S_SCALE * x * w[None, :])                   # (B, D)
    x2w = (x * x) @ w                                 # (B,)
    e2w = (e * e) @ w                                 # (N,)

    # xw_t[r, g, s, m] = S*xw[m, (2g+s)*128 + r]
    xw_t = np.ascontiguousarray(
        xw.T.reshape(NG, 2, 128, B).transpose(2, 0, 1, 3)).astype(fp8)

    per_core = []
    for cid in range(NCORES):
        sl = slice(cid * N_LOC, (cid + 1) * N_LOC)
        e_c = e[sl]                                   # (N_LOC, D)
        # e_t[g, r, s, n] = e_c[n, (2g+s)*128 + r]
        e_t = np.ascontiguousarray(
            e_c.T.reshape(NG, 2, 128, N_LOC).transpose(0, 2, 1, 3)).astype(fp8)
        # oh[r, s, p*CP + c]
        lab_c = labels[sl].reshape(NTILES, 128)       # (t, r)
        oh = np.zeros((128, 2, NPAIR * CP), dtype=np.float32)
        for p in range(NPAIR):
            for s in range(2):
                lab = lab_c[2 * p + s]                # (128,)
                oh[np.arange(128), s, p * CP + lab] = 1.0
        bias_c = (-beta * e2w[sl]).astype(np.float32).reshape(NTILES, 128).T
        per_core.append({
            "e_t": e_t,
            "xw_t": xw_t,
            "oh": oh.astype(fp8),
            "bias": np.ascontiguousarray(bias_c),
        })
    return per_core, beta, gamma, x2w


def kernel(x, ex_feats, ex_labels, w_unconstrained, gamma_unconstrained,
           beta_unconstrained, _want_results=False, **run_kwargs):
    from concourse.bass_utils import run_bass_kernel_spmd

    per_core, beta, gamma, x2w = _prepare(
        x, ex_feats, ex_labels, w_unconstrained, gamma_unconstrained,
        beta_unconstrained)

    key = round(beta, 12)
    if key not in _prog_cache:
        _prog_cache[key] = _build_program(beta)
    nc = _prog_cache[key]

    res = run_bass_kernel_spmd(nc, per_core, list(range(NCORES)), **run_kwargs)
    # Unshard: the per-class partial sums are associative -- sum the 8
    # partials, then apply the factored-out exp(-beta*x2w) and gamma*log.
    parts = np.zeros((CP, B), dtype=np.float64)
    for cid in range(NCORES):
        parts += np.asarray(res.results[cid]["cls"], dtype=np.float64)
    class_sum = parts[:C, :].T * np.exp(-beta * x2w)[:, None]   # (B, C)
    out = (gamma * np.log(class_sum + EPS)).astype(np.float32)
    if _want_results:
        return out, res
    return out
